# revision 78
# baseline (speedup 1.0000x reference)
"""Trainium2 Bass kernel for AdaptiveContourConv (B=4, 128->256ch, 64x64).

Sharding: 8 cores = batch(4) x H-half(2); each core computes output rows
[r0, r0+32) of one sample. All halos come from host-side sharding -> no
inter-core communication.

Per-core pipeline (matmuls in bf16 on the PE):
  conv1 (oc1|mc1 stacked)  -> h|hm            [PE + ACT(BN+ReLU)]
  conv2 (oc2|mc2 stacked)  -> offsets, mask   [PE + ACT]
  PE-transpose offs/mask to hw-major; bilinear corner idx + weights [DVE]
  per-kk: TWO dma_gathers (1216 idx each) from the pair-interleaved
          slab: each descriptor brings the full 2x2 corner patch (4
          pixels x 128ch = 1KB contiguous); one wide DVE mul applies
          premultiplied corner weights (mask folded); 2 tree adds fold
          the 4 corners; per-group PE transposes to channel-major; the
          kk's half-0 einsum matmuls accumulate into 5 persistent PSUM
          chunks (kk-major: einsum rides along the gathers) [Pool+DVE+PE]
  half-1 einsum after the loop (valT kept)                        [PE]
  contour: depthwise via diagonal matmuls + 1x1                   [PE]
  attention: M=49 (dx,dy) matmuls over 3 channel blocks; dx-fold on
     DVE (contiguous partition blocks); dy-fold via K=1 accumulated
     matmuls on contiguous shifted windows; sigmoid; replicate; attn
     multiply commuted past the fusion 1x1                   [PE+DVE]
  fusion matmul + BN + ReLU -> out                            [PE+ACT]
"""

import numpy as np

# ---------------- problem constants ----------------
B, C_IN, C_OUT, H, W, K = 4, 128, 256, 64, 64, 3
KK = K * K
MID = C_IN // 4
EPS = 1e-5

# ---------------- per-core geometry ----------------
R_OUT = 32          # output rows per core
R_CMB = 38          # main_feat/contour rows  [r0-3, r1+3)
R_HM = 40           # h/hm rows               [r0-4, r1+4)
R_X = 42            # c-major x slab rows     [r0-5, r1+5)
XC_W = 66           # padded width (+-1) for conv input
CMB_W = 70          # padded width (+-3) for combined (attn 7x7)
SLAB_R = 48         # pixel-major slab rows   [r0-8, r1+8)
SLAB_W = 72         # pixel-major slab cols   [-4, 68)
HW_CMB = R_CMB * W            # 2432
NS = KK * HW_CMB              # samples = 21888
NGK = HW_CMB // 128           # groups per kk = 19
N_PAIR_E = (SLAB_R // 2) * SLAB_W     # even row-pair entries = 1728
N_ENT = 2 * N_PAIR_E                  # total pair-slab entries = 3456
ACMB = R_OUT * CMB_W          # attn/fusion flat window = 2240


def _f32(x):
    return np.ascontiguousarray(np.asarray(x), dtype=np.float32)


def _bf16(x):
    import ml_dtypes
    return np.ascontiguousarray(
        np.asarray(x, dtype=np.float32).astype(ml_dtypes.bfloat16))


def build_bass(stage=99, loop_n=1, debug_out=False):
    import contextlib
    import concourse.bass as bass
    import concourse.mybir as mybir
    import concourse.tile as tile
    from concourse import bacc
    from concourse.ap import AP
    from concourse.masks import make_identity

    dt = mybir.dt
    Alu = mybir.AluOpType
    Act = mybir.ActivationFunctionType

    nc = bacc.Bacc("TRN2", target_bir_lowering=False, debug=False,
                   num_swdge_queues=4)

    # ---------------- DRAM parameters ----------------
    def P(name, shape, dtype=dt.bfloat16):
        return nc.declare_dram_parameter(name, shape, dtype, isOutput=False)

    xc_d = P("xc", [C_IN, 1 + R_X * XC_W + 1])        # c-major padded x slab
    xsp_d = P("xsp", [N_ENT + 1, 2 * C_IN])           # pair-interleaved slab
    w1_d = P("w1", [C_IN, KK, 64])                    # conv1 lhsT [c][tap][m]
    s1_d = P("s1", [64, 1], dt.float32)
    b1_d = P("b1", [64, 1], dt.float32)
    w2_d = P("w2", [64, KK, 41])                      # conv2 lhsT [c][tap][m]
    b2_d = P("b2", [41, 1], dt.float32)
    pyb_d = P("pyb", [128, KK, NGK], dt.float32)      # py base (slab coords)
    pxb_d = P("pxb", [128, KK, NGK], dt.float32)
    w2e_d = P("w2e", [C_IN, KK, C_OUT])               # einsum lhsT [c][kk][o]
    dcb_d = P("dcb", [128, 2], dt.float32)
    wdw_d = P("wdw", [C_IN, KK, C_IN])                # dw diag [c][tap][c']
    sdw_d = P("sdw", [C_IN, 1], dt.float32)
    bdw_d = P("bdw", [C_IN, 1], dt.float32)
    wpw_d = P("wpw", [C_IN, C_IN])                    # cb_pw lhsT [c][o]
    bpw_d = P("bpw", [C_IN, 1], dt.float32)
    saw_d = P("saw", [128, 3, 49])                    # sa lhsT [c][kt][dx*7+dy]
    fuw_d = P("fuw", [128, 3, C_OUT])                 # fu lhsT [c][kt][o]
    sf_d = P("sf", [128, 2], dt.float32)
    bf_d = P("bf", [128, 2], dt.float32)
    ones_d = P("ones1", [7, 128])
    mhm_d = P("mhm", [64, 8])                         # valid-row mask top4|bot4
    mcmb_d = P("mcmb", [128, 6])                      # valid-row mask top3|bot3

    out_d = nc.declare_dram_parameter("out", [C_OUT, R_OUT, W], dt.bfloat16,
                                      isOutput=True)
    dbg = {}
    if debug_out:
        for nm, shp, dty in [
                ("d_hm", [64, 1 + R_HM * XC_W + 1], dt.bfloat16),
                ("d_offmask", [41, HW_CMB], dt.bfloat16),
                ("d_idx", [128, KK * NGK], dt.float32),
                ("d_valT", [128, NS], dt.bfloat16),
                ("d_main0", [128, 3 + R_CMB * CMB_W + 3], dt.bfloat16),
                ("d_main1", [128, 3 + R_CMB * CMB_W + 3], dt.bfloat16),
                ("d_contour", [128, 3 + R_CMB * CMB_W + 3], dt.bfloat16),
                ("d_attn", [1, ACMB], dt.bfloat16)]:
            dbg[nm] = nc.declare_dram_parameter(nm, shp, dty,
                                                isOutput=True)

    with tile.TileContext(nc) as tc:
        with (
            tc.tile_pool(name="const", bufs=1) as const,
            tc.tile_pool(name="work", bufs=1) as work,
            tc.tile_pool(name="gath", bufs=8) as gath,
            tc.tile_pool(name="ps_m", bufs=7, space="PSUM") as ps_m,
            tc.tile_pool(name="ps_t", bufs=1, space="PSUM") as ps_t,
        ):
            _lp = tc.For_i(0, loop_n) if loop_n > 1 else \
                contextlib.nullcontext()
            with _lp:
                # ---------- load constants ----------
                def load(dram):
                    t = const.tile(list(dram.shape), dram.dtype,
                                   tag=dram.name + "_c", name=dram.name + "_c")
                    nc.sync.dma_start(t[:], dram[:])
                    return t

                xc = load(xc_d)
                w1 = load(w1_d)
                s1 = load(s1_d)
                b1 = load(b1_d)
                w2 = load(w2_d)
                b2 = load(b2_d)
                pyb = load(pyb_d)
                pxb = load(pxb_d)
                w2e = load(w2e_d)
                dcb = load(dcb_d)
                wdw = load(wdw_d)
                sdw = load(sdw_d)
                bdw = load(bdw_d)
                wpw = load(wpw_d)
                bpw = load(bpw_d)
                saw = load(saw_d)
                fuw = load(fuw_d)
                sf = load(sf_d)
                bf = load(bf_d)
                ones = load(ones_d)
                mhm = load(mhm_d)
                mcmb = load(mcmb_d)

                identb = const.tile([128, 128], dt.bfloat16, tag="identb")
                make_identity(nc, identb[:])

                # persistent PSUM tile helper for transposes (tag-shared
                # with conv chunk psums; rotates in ps_a's 2 bufs)
                def ps_cv():
                    return ps_m.tile([128, 512], dt.float32, tag="mm",
                                     name="ps_mm")

                def ps_tp():
                    return ps_t.tile([128, 512], dt.bfloat16, tag="tp",
                                     name="ps_tp")

                if stage >= 1:
                    # ---------- conv1: x -> h|hm (64ch, R_HM rows) ----------
                    # flat-contiguous rhs over the padded grid (strided matmul
                    # rhs hangs HW); pad columns compute garbage that
                    # epilogues skip.
                    xcf = xc[:]
                    hm = work.tile([64, 1 + R_HM * XC_W + 1], dt.bfloat16,
                                   tag="hm")
                    nc.gpsimd.memset(hm[:], 0.0)

                    def conv3x3(src_flat, src_w, lhsT_of, prow0, prow1, epi):
                        # out position p = prow*src_w + col (all cols); rhs
                        # flat slice = src_flat[, p + (ky-1)*src_w + (kx-1)]
                        # (+1 pad); chunk-pairs share one weight load per tap
                        chunks = []
                        r = prow0
                        while r < prow1:
                            nr = min(7, prow1 - r)
                            chunks.append((r, nr))
                            r += nr
                        pairs = [chunks[i:i + 2]
                                 for i in range(0, len(chunks), 2)]
                        for pair in pairs:
                            pss = [ps_cv() for _ in pair]
                            for t in range(KK):
                                ky, kx = t // 3, t % 3
                                for ci, (r0p, nr) in enumerate(pair):
                                    n = nr * src_w
                                    s0 = 1 + (r0p + ky - 1) * src_w \
                                        + (kx - 1)
                                    nc.tensor.matmul(
                                        pss[ci][:lhsT_of(t).shape[-1], :n],
                                        lhsT_of(t),
                                        src_flat[:, s0:s0 + n],
                                        start=(t == 0), stop=(t == KK - 1))
                            for ci, (r0p, nr) in enumerate(pair):
                                epi(r0p, nr, pss[ci])

                    def epi1(r0p, nr, ps):
                        base = 1 + (r0p - 1) * XC_W
                        nc.scalar.activation(
                            hm[:, base:base + nr * XC_W]
                            .rearrange("p (a b) -> p a b", b=XC_W)[:, :, 1:65],
                            ps[0:64, 0:nr * XC_W]
                            .rearrange("p (a b) -> p a b", b=XC_W)[:, :, 1:65],
                            Act.Relu, bias=b1[:], scale=s1[:])

                    conv3x3(xcf, XC_W, lambda t: w1[:, t, :], 1, R_X - 1, epi1)

                    # zero invalid halo rows (only ever in the top-4/bottom-4)
                    hmv = hm[:, 1:1 + R_HM * XC_W]\
                        .rearrange("p (a b) -> p a b", b=XC_W)[:, :, 1:65]
                    nc.vector.tensor_tensor(
                        hmv[:, 0:4], hmv[:, 0:4],
                        mhm[:, 0:4].unsqueeze(-1).broadcast_to([64, 4, 64]),
                        Alu.mult)
                    nc.vector.tensor_tensor(
                        hmv[:, R_HM - 4:R_HM], hmv[:, R_HM - 4:R_HM],
                        mhm[:, 4:8].unsqueeze(-1).broadcast_to([64, 4, 64]),
                        Alu.mult)

                    # ---------- conv2 -> offsets(18)|mask(9), R_CMB rows -----
                    # 48 partitions (xbar-transpose wants a multiple of 16);
                    # rows 41-47 are unused garbage
                    offmask = work.tile([48, HW_CMB], dt.bfloat16,
                                        tag="offmask")
                    nc.gpsimd.memset(offmask[32:48, :], 0.0)

                    def epi2(r0p, nr, ps):
                        j0 = r0p - 1          # offset-row index
                        pv = ps[:, 0:nr * XC_W]\
                            .rearrange("p (a b) -> p a b", b=XC_W)
                        nc.scalar.activation(
                            offmask[0:41, j0 * 64:(j0 + nr) * 64]
                            .rearrange("p (a b) -> p a b", b=64),
                            pv[0:41, :, 1:65],
                            Act.Identity, bias=b2[:], scale=1.0)

                    conv3x3(hm[:], XC_W, lambda t: w2[:, t, :], 1, 1 + R_CMB,
                            epi2)

                    # ---------- transpose offsets/mask to hw-major ----------
                    # xbar DMA transposes: toffs[p, g, c] =
                    # offmask[c, g*128 + p]; split so groups 0-6 (ready
                    # after ~2/6 of conv2's rows) unblock the first
                    # gathers early
                    toffs = work.tile([128, NGK, 48], dt.bfloat16,
                                      tag="toffs")
                    nc.sync.dma_start_transpose(toffs[:, 0:7, :],
                                                offmask[:, 0:7 * 128])
                    nc.sync.dma_start_transpose(toffs[:, 7:NGK, :],
                                                offmask[:, 7 * 128:HW_CMB])

                    def tof(c0, c1, step=1):
                        # [128, 9-ish, NGK] kk-major view of toffs cols
                        return toffs[:, :, c0:c1:step]\
                            .rearrange("p a b -> p b a")

                    offy = tof(0, 18, 2)            # [128, 9, NGK]
                    offx = tof(1, 18, 2)

                    # ---------- bilinear fields (hw-major) ----------
                    # index math first (unblocks the gathers), weights
                    # after (they overlap the first gathers)
                    fsh = [128, KK, NGK]

                    def ftile(name):
                        return work.tile(fsh, dt.float32, tag=name, name=name)

                    # tile buffers are reused across lifetimes via shared
                    # tags (work pool bufs=1 -> same buffer, WAR-tracked):
                    # fy reuses qf's, fx reuses parf's, ta py's, tb px's.
                    py, px = ftile("py"), ftile("px")
                    y0f, x0f = ftile("y0f"), ftile("x0f")
                    tu = ftile("tu")
                    qf, parf = ftile("qf"), ftile("parf")
                    idxf = ftile("idxf")
                    # corner weights: [kk][g][corner4: 00,10,01,11][dup-pair]
                    wsb = work.tile([128, KK, NGK, 4, 2], dt.bfloat16,
                                    tag="wsb")

                    # floor via fp-add magic: rint(v) = (v + 2^23) - 2^23 in
                    # f32 (round-nearest-even, identical on HW and in numpy);
                    # floor(py) = rint(py - 0.5) for py >= 0.  The int16 cast
                    # then converts an exact integer (rounding-mode-proof).
                    # The whole index chain runs per group-half (0:7 /
                    # 7:19) so the first gathers can launch while conv2's
                    # later rows are still in flight.
                    MAGIC = float(1 << 23)
                    idxwA = work.tile([128, KK, 7, 8], dt.int16, tag="idxwA")
                    idxwB = work.tile([128, KK, NGK - 7, 8], dt.int16,
                                      tag="idxwB")
                    for (a, b, idxw) in [(0, 7, idxwA), (7, NGK, idxwB)]:
                        def S(t):
                            return t[:, :, a:b]
                        nc.vector.tensor_tensor(S(py), offy[:, :, a:b],
                                                S(pyb), Alu.add)
                        nc.vector.tensor_scalar(S(py), S(py), 0.0,
                                                float(SLAB_R - 2),
                                                Alu.max, Alu.min)
                        nc.vector.tensor_scalar(S(tu), S(py), 0.5, MAGIC,
                                                Alu.subtract, Alu.add)
                        nc.vector.tensor_scalar(S(y0f), S(tu), MAGIC, None,
                                                Alu.subtract)

                        nc.vector.tensor_tensor(S(px), offx[:, :, a:b],
                                                S(pxb), Alu.add)
                        nc.vector.tensor_scalar(S(px), S(px), 0.0,
                                                float(SLAB_W - 2),
                                                Alu.max, Alu.min)
                        nc.vector.tensor_scalar(S(tu), S(px), 0.5, MAGIC,
                                                Alu.subtract, Alu.add)
                        nc.vector.tensor_scalar(S(x0f), S(tu), MAGIC, None,
                                                Alu.subtract)

                        # pair-slab entry index:
                        #   q = floor(y0/2) = rint(y0*0.5 - 0.25)  (exact
                        #       for integer y0; 0.25 keeps half-integers
                        #       off the round-to-even boundary)
                        #   parity = y0 - 2q
                        #   eidx = (q + parity*1728/72)*72 + x0
                        nc.vector.tensor_scalar(S(tu), S(y0f), 0.5, 0.25,
                                                Alu.mult, Alu.subtract)
                        nc.vector.tensor_scalar(S(qf), S(tu), MAGIC, MAGIC,
                                                Alu.add, Alu.subtract)
                        nc.vector.scalar_tensor_tensor(
                            S(parf), S(qf), -2.0, S(y0f), Alu.mult, Alu.add)
                        nc.vector.scalar_tensor_tensor(
                            S(tu), S(parf), float(N_PAIR_E // SLAB_W),
                            S(qf), Alu.mult, Alu.add)
                        nc.vector.scalar_tensor_tensor(
                            S(idxf), S(tu), float(SLAB_W), S(x0f),
                            Alu.mult, Alu.add)
                        idxi = work.tile([128, KK, b - a], dt.int16,
                                         tag=f"idxi{a}")
                        nc.vector.tensor_copy(idxi[:], S(idxf))

                        # wrap + replicate indices for dma_gather:
                        # idxw[16r+p16, kk, g, q] = idx of sample g*128 +
                        # 16q + p16; the gather's j-th index lives at
                        # partition j%16, free j//16.
                        for q in range(8):
                            nc.sync.dma_start(idxw[0:16, :, :, q],
                                              idxi[16 * q:16 * (q + 1)])
                        nc.sync.dma_start(idxw[16:32], idxw[0:16])
                        nc.sync.dma_start(idxw[32:64], idxw[0:32])
                        nc.sync.dma_start(idxw[64:128], idxw[0:64])

                    # fractional parts + mask (overlap the first gathers)
                    fy, fx = ftile("qf"), ftile("parf")
                    nc.vector.tensor_tensor(fy[:], py[:], y0f[:],
                                            Alu.subtract)
                    nc.vector.tensor_tensor(fx[:], px[:], x0f[:],
                                            Alu.subtract)
                    msk2 = work.tile([128, KK, NGK], dt.float32, tag="msk2")
                    nc.scalar.activation(msk2[:], tof(32, 41), Act.Sigmoid)
                    maskT = msk2[:]

                    # corner weights (mask folded), order (00, 10, 01, 11)
                    # matching the gathered pair-slab patch layout
                    # [v00 v10 v01 v11]: w00=m(1-fy)(1-fx), w10=m*fy(1-fx),
                    # w01=m(1-fy)fx, w11=m*fy*fx.  Each weight is stored as
                    # an adjacent duplicated pair so the mul's weight operand
                    # has a packed last dim (stride 1, count 2) -> DVE 2x
                    # mode applies despite the broadcast.
                    ta, tb = ftile("py"), ftile("px")
                    nc.vector.tensor_tensor(tb[:], maskT, fy[:], Alu.mult)
                    nc.vector.tensor_tensor(ta[:], maskT, tb[:],
                                            Alu.subtract)
                    nc.vector.tensor_tensor(tu[:], ta[:], fx[:], Alu.mult)
                    for j in range(2):
                        nc.vector.tensor_copy(wsb[:, :, :, 2, j], tu[:])
                    nc.vector.tensor_tensor(tu[:], ta[:], tu[:],
                                            Alu.subtract)
                    for j in range(2):
                        nc.vector.tensor_copy(wsb[:, :, :, 0, j], tu[:])
                    nc.vector.tensor_tensor(tu[:], tb[:], fx[:], Alu.mult)
                    for j in range(2):
                        nc.vector.tensor_copy(wsb[:, :, :, 3, j], tu[:])
                    nc.vector.tensor_tensor(tu[:], tb[:], tu[:],
                                            Alu.subtract)
                    for j in range(2):
                        nc.vector.tensor_copy(wsb[:, :, :, 1, j], tu[:])

                    def cmb_grid(t):      # [128, R_CMB, CMB_W] view of flat
                        return t[:, 3:3 + R_CMB * CMB_W]\
                            .rearrange("p (a b) -> p a b", b=CMB_W)

                    mm_chunks = [(0, 512), (512, 512), (1024, 512),
                                 (1536, 512), (2048, 384)]

                if stage >= 2:
                    # ---------- contour branch ----------
                    hc = work.tile([C_IN, HW_CMB], dt.bfloat16, tag="hc")

                    def epi_dw(r0p, nr, ps):
                        j0 = r0p - 2
                        nc.scalar.activation(
                            hc[:, j0 * 64:(j0 + nr) * 64]
                            .rearrange("p (a b) -> p a b", b=64),
                            ps[:, 0:nr * XC_W]
                            .rearrange("p (a b) -> p a b", b=XC_W)[:, :, 1:65],
                            Act.Relu, bias=bdw[:], scale=sdw[:])

                    conv3x3(xcf, XC_W, lambda t: wdw[:, t, :], 2, 2 + R_CMB,
                            epi_dw)
                    contour = work.tile([C_IN, 3 + R_CMB * CMB_W + 3],
                                        dt.bfloat16, tag="contour")
                    nc.gpsimd.memset(contour[:], 0.0)
                    for (c0, cn) in mm_chunks:
                        ps = ps_cv()
                        nc.tensor.matmul(ps[:, :cn], wpw[:], hc[:, c0:c0 + cn],
                                         start=True, stop=True)
                        r0, nr = c0 // 64, cn // 64
                        nc.scalar.activation(
                            cmb_grid(contour)[:, r0:r0 + nr, 3:67],
                            ps[:, :cn].rearrange("p (a b) -> p a b", b=64),
                            Act.Identity, bias=bpw[:], scale=1.0)

                if stage >= 3:
                    # ---------- per-kk: gather, weight, fold, transpose,
                    # ---------- and half-0 einsum (kk-major) ----------
                    xsp_flat = AP(tensor=xsp_d, offset=0,
                                  ap=[[2 * C_IN, N_ENT], [1, 4 * C_IN]])
                    valT = work.tile([128, 1, NS], dt.bfloat16, tag="valT")

                    main_sb = []
                    for hf in range(2):
                        m_t = work.tile([128, 3 + R_CMB * CMB_W + 3],
                                        dt.bfloat16, tag=f"main{hf}")
                        nc.gpsimd.memset(m_t[:], 0.0)
                        main_sb.append(m_t)

                    # persistent kk-major einsum accumulators: half 0 all 5
                    # chunks + half 1 chunks 0-1 (7 of the 7 ps_m bufs);
                    # half-1 chunks 2-4 run post-loop.
                    ek = [(0, c0, cn) for (c0, cn) in mm_chunks] + \
                        [(1, c0, cn) for (c0, cn) in mm_chunks[:2]]
                    pse0 = {}
                    for (hf, c0, cn) in ek:
                        pse0[(hf, c0)] = ps_m.tile([128, 512], dt.float32,
                                                   tag="mm", name="mm")

                    # <=1024 descriptors per gather: the 16KB SWDGE
                    # descriptor carveout (16B/desc) caps one instruction
                    # at ~1024 on HW.  Splits align with the idxwA/idxwB
                    # group halves.
                    splits = [(0, 7, 0), (7, 6, 0), (13, 6, 6)]
                    for kk_i in range(KK):
                        halves = []
                        for gi, (g0, ng, grel) in enumerate(splits):
                            idxw = idxwA if g0 < 7 else idxwB
                            gtb = gath.tile([128, 7, 512], dt.bfloat16,
                                            tag="gtb")
                            halves.append((g0, ng, gtb))
                            ni = ng * 128
                            nc.gpsimd.dma_gather(
                                gtb[:, 0:ng, :], xsp_flat,
                                idxw[:, kk_i, grel:grel + ng, :], ni, ni,
                                elem_size=512, elem_step=2 * C_IN,
                                queue_num=(3 * kk_i + gi) % 4)

                        # corner weights in place (paired dup weights ->
                        # packed last dim -> DVE 2x), then fold the 4
                        # corners with 2 tree adds (packed, 2x)
                        for (g0, ng, gtb) in halves:
                            g_v = gtb[:, 0:ng, :]\
                                .rearrange("p a (b c d) -> p a b c d",
                                           b=4, d=2)
                            w_v = wsb[:, kk_i, g0:g0 + ng].unsqueeze(3)\
                                .broadcast_to([128, ng, 4, 64, 2])
                            nc.vector.tensor_tensor(g_v, g_v, w_v, Alu.mult)
                            nc.vector.tensor_tensor(
                                gtb[:, 0:ng, 0:256], gtb[:, 0:ng, 0:256],
                                gtb[:, 0:ng, 256:512], Alu.add)
                            nc.vector.tensor_tensor(
                                gtb[:, 0:ng, 0:128], gtb[:, 0:ng, 0:128],
                                gtb[:, 0:ng, 128:256], Alu.add)

                        # transpose each 128-sample group to channel-major
                        for gch in range(5):      # 4+4+4+4+3 groups of 128
                            nu = 4 if gch < 4 else 3
                            pst = ps_tp()
                            for u in range(nu):
                                g = gch * 4 + u
                                for (g0, ng, t_) in halves:
                                    if g0 <= g < g0 + ng:
                                        gtb = t_
                                        gl = g - g0
                                        break
                                nc.tensor.transpose(
                                    pst[:, u * 128:(u + 1) * 128],
                                    gtb[:, gl, 0:128], identb[:])
                            nc.scalar.activation(
                                valT[:, 0, kk_i * HW_CMB + gch * 512:
                                     kk_i * HW_CMB + gch * 512 + nu * 128],
                                pst[:, :nu * 128], Act.Copy)

                        # in-loop einsum contributions of this kk
                        for (hf, c0, cn) in ek:
                            rhs = valT[:, 0, kk_i * HW_CMB + c0:
                                       kk_i * HW_CMB + c0 + cn]
                            nc.tensor.matmul(
                                pse0[(hf, c0)][:, :cn],
                                w2e[:, kk_i, hf * 128:(hf + 1) * 128], rhs,
                                start=(kk_i == 0), stop=(kk_i == KK - 1))

                    def epi_main(hf, c0, cn, ps):
                        r0, nr = c0 // 64, cn // 64
                        nc.scalar.activation(
                            cmb_grid(main_sb[hf])[:, r0:r0 + nr, 3:67],
                            ps[:, :cn].rearrange("p (a b) -> p a b", b=64),
                            Act.Identity, bias=dcb[:, hf:hf + 1], scale=1.0)

                    for (hf, c0, cn) in ek:
                        epi_main(hf, c0, cn, pse0[(hf, c0)])

                    # ---------- remaining einsum: half-1 chunks 2-4 ----------
                    # emission deferred (stage>=5 weaves it between the
                    # first attention chunks, which only need in-loop
                    # einsum results)
                    def rest_einsum():
                        rest = [(1, c0, cn) for (c0, cn) in mm_chunks[2:]]
                        psr = {}
                        for (hf, c0, cn) in rest:
                            psr[(hf, c0)] = ps_m.tile([128, 512],
                                                      dt.float32,
                                                      tag="mm", name="mm")
                        for kk_i in range(KK):
                            for (hf, c0, cn) in rest:
                                rhs = valT[:, 0, kk_i * HW_CMB + c0:
                                           kk_i * HW_CMB + c0 + cn]
                                nc.tensor.matmul(
                                    psr[(hf, c0)][:, :cn],
                                    w2e[:, kk_i, hf * 128:(hf + 1) * 128],
                                    rhs, start=(kk_i == 0),
                                    stop=(kk_i == KK - 1))
                        for (hf, c0, cn) in rest:
                            epi_main(hf, c0, cn, psr[(hf, c0)])

                    # ---------- zero invalid rows (only top-3/bottom-3) -----
                    cmb = [main_sb[0], main_sb[1], contour]

                    def zero_rows(r0z, msl):
                        for cti in range(3):
                            ctv = cmb_grid(cmb[cti])[:, :, 3:67]
                            nc.vector.tensor_tensor(
                                ctv[:, r0z:r0z + 3], ctv[:, r0z:r0z + 3],
                                mcmb[:, msl:msl + 3].unsqueeze(-1)
                                .broadcast_to([128, 3, 64]), Alu.mult)

                    zero_rows(0, 0)           # top rows: in-loop chunks
                    if stage < 5:
                        rest_einsum()
                        zero_rows(R_CMB - 3, 3)

                if stage >= 5:
                    # ---------- attention: 7x7 conv -> 1 channel ----------
                    # pm49[dx*7+dy, j*70+c'] = sum_c saw[c,kt,dx*7+dy]*cmb
                    # over kt blocks; then fold dx (PE, shifted), fold dy
                    # (PE, K=1 accumulated matmuls on contiguous windows),
                    # sigmoid.  Emission is software-pipelined per chunk so
                    # the 4 PE stages and their ACT copies overlap.
                    NPM = R_CMB * CMB_W                     # 2660
                    NP7 = NPM - 6
                    pm49 = work.tile([49, NPM], dt.bfloat16, tag="pm49")
                    pm7 = work.tile([7, NPM], dt.bfloat16, tag="pm7")
                    nc.gpsimd.memset(pm7[:, NP7:NPM], 0.0)
                    attn = work.tile([1, ACMB], dt.bfloat16, tag="attn")
                    attn_r = work.tile([128, ACMB], dt.bfloat16,
                                       tag="attn_r")
                    a_chunks = [(0, 448), (448, 448), (896, 448),
                                (1344, 448), (1792, 448), (2240, 420)]
                    x_chunks = [(0, 448), (448, 448), (896, 448), (1344, 448),
                                (1792, 448), (2240, NP7 - 2240)]
                    f_chunks = [(0, 448), (448, 448), (896, 448),
                                (1344, 448), (1792, 448)]

                    def do_pm49(i0, n):
                        ps = ps_cv()
                        for kt in range(3):
                            nc.tensor.matmul(ps[0:49, :n], saw[:, kt, :],
                                             cmb[kt][:, 3 + i0:3 + i0 + n],
                                             start=(kt == 0), stop=(kt == 2))
                        nc.scalar.activation(pm49[:, i0:i0 + n],
                                             ps[0:49, :n], Act.Copy)

                    def do_dx(i0, n):
                        # pm7[dy, q] = sum_dx pm49[dx*7+dy, q+dx]
                        ps = ps_cv()
                        for dx in range(7):
                            nc.tensor.matmul(
                                ps[0:7, :n], identb[0:49, 7 * dx:7 * dx + 7],
                                pm49[:, i0 + dx:i0 + dx + n],
                                start=(dx == 0), stop=(dx == 6))
                        nc.scalar.activation(pm7[:, i0:i0 + n], ps[0:7, :n],
                                             Act.Copy)

                    def do_dy(i0, n):
                        # attn[q=r*70+c'] = sig(sum_dy pm7[dy, q + dy*70])
                        ps = ps_cv()
                        for dy in range(7):
                            nc.tensor.matmul(
                                ps[0:1, :n], identb[0:7, dy:dy + 1],
                                pm7[0:7, dy * CMB_W + i0:dy * CMB_W + i0 + n],
                                start=(dy == 0), stop=(dy == 6))
                        nc.scalar.activation(attn[:, i0:i0 + n], ps[0:1, :n],
                                             Act.Sigmoid)

                    def do_rep(i0, n):
                        # replicate attn to 128 partitions via K=1 matmul
                        ps = ps_cv()
                        nc.tensor.matmul(ps[:, :n], ones[0:1, :],
                                         attn[:, i0:i0 + n],
                                         start=True, stop=True)
                        nc.scalar.activation(attn_r[:, i0:i0 + n], ps[:, :n],
                                             Act.Copy)

                    # fusion 1x1 matmuls are independent of attn: woven in
                    # to keep the PE busy during the fold ACT copies
                    fvts = []
                    fu_work = []
                    if stage >= 6:
                        for hf in range(2):
                            fvt = work.tile([128, ACMB], dt.bfloat16,
                                            tag=f"fvt{hf}")
                            fvts.append(fvt)

                        def do_fu(hf, i0, n):
                            ps = ps_cv()
                            for kt in range(3):
                                rhs = cmb[kt][:, 3 + 3 * CMB_W + i0:
                                              3 + 3 * CMB_W + i0 + n]
                                nc.tensor.matmul(
                                    ps[:, :n],
                                    fuw[:, kt, hf * 128:(hf + 1) * 128],
                                    rhs, start=(kt == 0), stop=(kt == 2))
                            nc.scalar.activation(fvts[hf][:, i0:i0 + n],
                                                 ps[:, :n], Act.Copy)

                        fu_work = [(hf, i0, n) for hf in range(2)
                                   for (i0, n) in f_chunks]

                    # software-pipelined emission: pm49 chunks 0-1 only
                    # need in-loop einsum output, so they run BEFORE the
                    # remaining einsum chunks; then dx chunk i needs pm49
                    # through chunk i+1, dy chunk i needs pm7 through
                    # chunk i+1, rep chunk i needs dy chunk i.
                    do_pm49(*a_chunks[0])
                    do_pm49(*a_chunks[1])
                    rest_einsum()
                    zero_rows(R_CMB - 3, 3)
                    prog = [("dx", 0), ("pm49", 2), ("fu", 0), ("dx", 1),
                            ("pm49", 3), ("dy", 0), ("fu", 1), ("pm49", 4),
                            ("dx", 2), ("dy", 1), ("rep", 0), ("fu", 2),
                            ("pm49", 5), ("dx", 3), ("dy", 2), ("rep", 1),
                            ("fu", 3), ("dx", 4), ("dy", 3), ("rep", 2),
                            ("fu", 4), ("dx", 5), ("dy", 4), ("rep", 3),
                            ("fu", 5), ("rep", 4)]
                    nfu = len(fu_work)
                    for it in prog:
                        kind, arg = it
                        if kind == "pm49":
                            do_pm49(*a_chunks[arg])
                        elif kind == "dx":
                            do_dx(*x_chunks[arg])
                        elif kind == "dy":
                            do_dy(*f_chunks[arg])
                        elif kind == "rep":
                            do_rep(*f_chunks[arg])
                        elif kind == "fu":
                            if arg < nfu:
                                do_fu(*fu_work[arg])
                    for w_ in fu_work[6:]:
                        do_fu(*w_)

                if stage >= 6:
                    # ---------- fusion epilogue: attn multiply + out --------
                    # row-halved so the first half pipelines behind the
                    # attn-replicate chunks (slice-granular deps)
                    for hf in range(2):
                        fvt = fvts[hf]
                        fm = work.tile([128, R_OUT, 64], dt.bfloat16,
                                       tag="fm")
                        outt = work.tile([128, R_OUT * W], dt.bfloat16,
                                         tag="outt")
                        ov = outt[:].rearrange("p (a b) -> p a b", b=64)
                        for (r0h, nrh) in [(0, 16), (16, 16)]:
                            # attn multiply: 70-col layout, 3-col offset
                            nc.vector.tensor_tensor(
                                fm[:, r0h:r0h + nrh],
                                fvt[:].rearrange(
                                    "p (a b) -> p a b",
                                    b=CMB_W)[:, r0h:r0h + nrh, 3:67],
                                attn_r[:].rearrange(
                                    "p (a b) -> p a b",
                                    b=CMB_W)[:, r0h:r0h + nrh, 0:64],
                                Alu.mult)
                            nc.scalar.activation(
                                ov[:, r0h:r0h + nrh],
                                fm[:, r0h:r0h + nrh], Act.Relu,
                                bias=bf[:, hf:hf + 1],
                                scale=sf[:, hf:hf + 1])
                            nc.sync.dma_start(
                                out_d[hf * 128:(hf + 1) * 128,
                                      r0h:r0h + nrh, :],
                                ov[:, r0h:r0h + nrh])

                if stage < 6:
                    o = work.tile([128, 16], dt.bfloat16, tag="stub")
                    nc.gpsimd.memset(o[:], 0.0)
                    for hf in range(2):
                        nc.sync.dma_start(
                            out_d[hf * 128:(hf + 1) * 128, 0:1, 0:16],
                            o[:].rearrange("p (a b) -> p a b", b=16))

                if debug_out:
                    def dump(nm, ap):
                        if len(ap.shape) > 2:
                            ap = ap.rearrange("p ... -> p (...)")
                        nc.sync.dma_start(dbg[nm][:], ap)
                    dump("d_hm", hm[:])
                    dump("d_offmask", offmask[:])
                    dump("d_idx", idxf[:])
                    dump("d_valT", valT[:])
                    dump("d_main0", main_sb[0][:])
                    dump("d_main1", main_sb[1][:])
                    dump("d_contour", contour[:])
                    dump("d_attn", attn[:])

    nc.compile()
    return nc


# ---------------- host-side input prep ----------------

def prep_core_inputs(d, core_id):
    b, half = core_id // 2, core_id % 2
    r0 = half * R_OUT

    x = _f32(d["x"][b])                       # [C_IN, H, W]

    xcg = np.zeros((C_IN, R_X, XC_W), np.float32)
    lo, hi = r0 - 5, r0 + R_OUT + 5
    slo, shi = max(lo, 0), min(hi, H)
    xcg[:, slo - lo:shi - lo, 1:65] = x[:, slo:shi, :]
    xc = np.zeros((C_IN, 1 + R_X * XC_W + 1), np.float32)
    xc[:, 1:1 + R_X * XC_W] = xcg.reshape(C_IN, -1)

    # pair-interleaved slab: entry (pair p, col c) holds the two pixels
    # (2p + parity, c), (2p + 1 + parity, c) with 128ch each; even copy
    # (parity 0) at entries [0, 1728), odd copy at [1728, 3456).
    slab = np.zeros((SLAB_R + 1, SLAB_W, C_IN), np.float32)
    lo2, hi2 = r0 - 8, r0 + R_OUT + 8
    slo2, shi2 = max(lo2, 0), min(hi2, H)
    slab[slo2 - lo2:shi2 - lo2, 4:68, :] = \
        x[:, slo2:shi2, :].transpose(1, 2, 0)
    xsp = np.zeros((N_ENT + 1, 2 * C_IN), np.float32)
    ev = xsp[:N_PAIR_E].reshape(SLAB_R // 2, SLAB_W, 2, C_IN)
    ev[:, :, 0] = slab[0:SLAB_R:2, :, :]
    ev[:, :, 1] = slab[1:SLAB_R + 1:2, :, :]
    od = xsp[N_PAIR_E:N_ENT].reshape(SLAB_R // 2, SLAB_W, 2, C_IN)
    od[:, :, 0] = slab[1:SLAB_R:2, :, :]
    od[:, :, 1] = slab[2:SLAB_R + 1:2, :, :]

    w1 = np.zeros((C_IN, KK, 64), np.float32)
    for t in range(KK):
        ky, kx = t // 3, t % 3
        w1[:, t, 0:32] = d["oc1_w"][:, :, ky, kx].T
        w1[:, t, 32:64] = d["mc1_w"][:, :, ky, kx].T
    sc_o = d["obn_g"] / np.sqrt(d["obn_v"] + EPS)
    bi_o = (d["oc1_b"] - d["obn_m"]) * sc_o + d["obn_b"]
    sc_m = d["mbn_g"] / np.sqrt(d["mbn_v"] + EPS)
    bi_m = (d["mc1_b"] - d["mbn_m"]) * sc_m + d["mbn_b"]
    s1 = np.concatenate([sc_o, sc_m])[:, None]
    b1 = np.concatenate([bi_o, bi_m])[:, None]

    w2 = np.zeros((64, KK, 41), np.float32)
    for t in range(KK):
        ky, kx = t // 3, t % 3
        w2[0:32, t, 0:18] = d["oc2_w"][:, :, ky, kx].T
        w2[32:64, t, 32:41] = d["mc2_w"][:, :, ky, kx].T
    b2 = np.zeros((41, 1), np.float32)
    b2[0:18, 0] = d["oc2_b"]
    b2[32:41, 0] = d["mc2_b"]

    kk = np.arange(KK)
    hw = np.arange(HW_CMB)
    r_i, w_i = hw // 64, hw % 64
    pyb = (r_i[None, :] + 4 + (kk // 3)[:, None]).astype(np.float32)
    pxb = (w_i[None, :] + 3 + (kk % 3)[:, None]).astype(np.float32)
    pyb = pyb.reshape(KK, NGK, 128).transpose(2, 0, 1)
    pxb = pxb.reshape(KK, NGK, 128).transpose(2, 0, 1)

    w2e = d["dc_w"].reshape(C_OUT, C_IN, KK).transpose(1, 2, 0)

    wdw = np.zeros((C_IN, KK, C_IN), np.float32)
    for t in range(KK):
        ky, kx = t // 3, t % 3
        np.fill_diagonal(wdw[:, t, :], d["cb_dw_w"][:, 0, ky, kx])
    sc_c = d["cbn_g"] / np.sqrt(d["cbn_v"] + EPS)
    bi_c = (d["cb_dw_b"] - d["cbn_m"]) * sc_c + d["cbn_b"]

    wpw = d["cb_pw_w"][:, :, 0, 0].T
    # saw[c, kt, dx*7+dy] = sa_w[0, kt*128+c, dy, dx]
    saw = d["sa_w"][0].reshape(3, 128, 7, 7).transpose(1, 0, 3, 2)\
        .reshape(128, 3, 49)
    fuw = d["fu_w"][:, :, 0, 0].T.reshape(3, 128, C_OUT).transpose(1, 0, 2)
    sc_f = d["fbn_g"] / np.sqrt(d["fbn_v"] + EPS)
    bi_f = (d["fu_b"] - d["fbn_m"]) * sc_f + d["fbn_b"]

    rows_hm = np.arange(r0 - 4, r0 + R_OUT + 4)
    vhm = ((rows_hm >= 0) & (rows_hm < H)).astype(np.float32)
    mhm = np.broadcast_to(np.concatenate([vhm[0:4], vhm[-4:]]),
                          (64, 8)).copy()
    rows_cmb = np.arange(r0 - 3, r0 + R_OUT + 3)
    vcmb = ((rows_cmb >= 0) & (rows_cmb < H)).astype(np.float32)
    mcmb = np.broadcast_to(np.concatenate([vcmb[0:3], vcmb[-3:]]),
                           (128, 6)).copy()

    return {
        "xc": _bf16(xc), "xsp": _bf16(xsp),
        "w1": _bf16(w1), "s1": _f32(s1), "b1": _f32(b1),
        "w2": _bf16(w2), "b2": _f32(b2),
        "pyb": _f32(pyb), "pxb": _f32(pxb),
        "w2e": _bf16(w2e), "dcb": _f32(d["dc_b"].reshape(2, 128).T),
        "wdw": _bf16(wdw), "sdw": _f32(sc_c[:, None]),
        "bdw": _f32(bi_c[:, None]),
        "wpw": _bf16(wpw), "bpw": _f32(d["cb_pw_b"][:, None]),
        "saw": _bf16(saw), "fuw": _bf16(fuw),
        "sf": _f32(sc_f.reshape(2, 128).T), "bf": _f32(bi_f.reshape(2, 128).T),
        "ones1": _bf16(np.ones((7, 128), np.float32)),
        "mhm": _bf16(mhm), "mcmb": _bf16(mcmb),
    }


_NC_CACHE = {}


def get_nc():
    if "nc" not in _NC_CACHE:
        _NC_CACHE["nc"] = build_bass()
    return _NC_CACHE["nc"]


def kernel(**inputs):
    from concourse.bass_utils import run_bass_kernel_spmd

    nc = get_nc()
    d = {k: np.asarray(v) for k, v in inputs.items()}
    in_maps = [prep_core_inputs(d, c) for c in range(8)]
    res = run_bass_kernel_spmd(nc, in_maps, core_ids=list(range(8)))

    out = np.zeros((B, C_OUT, H, W), np.float32)
    for c in range(8):
        b, half = c // 2, c % 2
        out[b, :, half * R_OUT:(half + 1) * R_OUT, :] = \
            np.asarray(res.results[c]["out"], dtype=np.float32)
    return out


# revision 82
# speedup vs baseline: 1.1698x; 1.1698x over previous
"""Trainium2 Bass kernel for AdaptiveContourConv (B=4, 128->256ch, 64x64).

Sharding: 8 cores = batch(4) x H-half(2); each core computes output rows
[r0, r0+32) of one sample. All halos come from host-side sharding -> no
inter-core communication.

Per-core pipeline (matmuls in bf16 on the PE):
  conv1 (oc1|mc1 stacked)  -> h|hm            [PE + ACT(BN+ReLU)]
  conv2 (oc2|mc2 stacked)  -> offsets, mask   [PE + ACT]
  PE-transpose offs/mask to hw-major; bilinear corner idx + weights [DVE]
  per-kk: TWO dma_gathers (1216 idx each) from the pair-interleaved
          slab: each descriptor brings the full 2x2 corner patch (4
          pixels x 128ch = 1KB contiguous); one wide DVE mul applies
          premultiplied corner weights (mask folded); 2 tree adds fold
          the 4 corners; per-group PE transposes to channel-major; the
          kk's half-0 einsum matmuls accumulate into 5 persistent PSUM
          chunks (kk-major: einsum rides along the gathers) [Pool+DVE+PE]
  half-1 einsum after the loop (valT kept)                        [PE]
  contour: depthwise via diagonal matmuls + 1x1                   [PE]
  attention: M=49 (dx,dy) matmuls over 3 channel blocks; dx-fold on
     DVE (contiguous partition blocks); dy-fold via K=1 accumulated
     matmuls on contiguous shifted windows; sigmoid; replicate; attn
     multiply commuted past the fusion 1x1                   [PE+DVE]
  fusion matmul + BN + ReLU -> out                            [PE+ACT]
"""

import numpy as np

# ---------------- problem constants ----------------
B, C_IN, C_OUT, H, W, K = 4, 128, 256, 64, 64, 3
KK = K * K
MID = C_IN // 4
EPS = 1e-5

# ---------------- per-core geometry ----------------
R_OUT = 32          # output rows per core
R_CMB = 38          # main_feat/contour rows  [r0-3, r1+3)
R_HM = 40           # h/hm rows               [r0-4, r1+4)
R_X = 42            # c-major x slab rows     [r0-5, r1+5)
XC_W = 66           # padded width (+-1) for conv input
CMB_W = 70          # padded width (+-3) for combined (attn 7x7)
SLAB_R = 48         # pixel-major slab rows   [r0-8, r1+8)
SLAB_W = 72         # pixel-major slab cols   [-4, 68)
HW_CMB = R_CMB * W            # 2432
NS = KK * HW_CMB              # samples = 21888
NGK = HW_CMB // 128           # groups per kk = 19
N_PAIR_E = (SLAB_R // 2) * SLAB_W     # even row-pair entries = 1728
N_ENT = 2 * N_PAIR_E                  # total pair-slab entries = 3456
ACMB = R_OUT * CMB_W          # attn/fusion flat window = 2240


def _f32(x):
    return np.ascontiguousarray(np.asarray(x), dtype=np.float32)


def _bf16(x):
    import ml_dtypes
    return np.ascontiguousarray(
        np.asarray(x, dtype=np.float32).astype(ml_dtypes.bfloat16))


def build_bass(stage=99, loop_n=1, debug_out=False):
    import contextlib
    import concourse.bass as bass
    import concourse.mybir as mybir
    import concourse.tile as tile
    from concourse import bacc
    from concourse.ap import AP
    from concourse.masks import make_identity

    dt = mybir.dt
    Alu = mybir.AluOpType
    Act = mybir.ActivationFunctionType

    nc = bacc.Bacc("TRN2", target_bir_lowering=False, debug=False,
                   num_swdge_queues=4)

    # ---------------- DRAM parameters ----------------
    def P(name, shape, dtype=dt.bfloat16):
        return nc.declare_dram_parameter(name, shape, dtype, isOutput=False)

    xc_d = P("xc", [C_IN, 1 + R_X * XC_W + 1])        # c-major padded x slab
    xsp_d = P("xsp", [N_ENT + 1, 2 * C_IN])           # pair-interleaved slab
    w1_d = P("w1", [C_IN, KK, 64])                    # conv1 lhsT [c][tap][m]
    s1_d = P("s1", [64, 1], dt.float32)
    b1_d = P("b1", [64, 1], dt.float32)
    w2_d = P("w2", [64, KK, 41])                      # conv2 lhsT [c][tap][m]
    b2_d = P("b2", [41, 1], dt.float32)
    pyb_d = P("pyb", [128, KK, NGK], dt.float32)      # py base (slab coords)
    pxb_d = P("pxb", [128, KK, NGK], dt.float32)
    w2e_d = P("w2e", [C_IN, KK, C_OUT])               # einsum lhsT [c][kk][o]
    dcb_d = P("dcb", [128, 2], dt.float32)
    wdw_d = P("wdw", [C_IN, KK, C_IN])                # dw diag [c][tap][c']
    sdw_d = P("sdw", [C_IN, 1], dt.float32)
    bdw_d = P("bdw", [C_IN, 1], dt.float32)
    wpw_d = P("wpw", [C_IN, C_IN])                    # cb_pw lhsT [c][o]
    bpw_d = P("bpw", [C_IN, 1], dt.float32)
    saw_d = P("saw", [128, 3, 49])                    # sa lhsT [c][kt][dx*7+dy]
    fuw_d = P("fuw", [128, 3, C_OUT])                 # fu lhsT [c][kt][o]
    sf_d = P("sf", [128, 2], dt.float32)
    bf_d = P("bf", [128, 2], dt.float32)
    ones_d = P("ones1", [7, 128])
    mhm_d = P("mhm", [64, 8])                         # valid-row mask top4|bot4
    mcmb_d = P("mcmb", [128, 6])                      # valid-row mask top3|bot3

    out_d = nc.declare_dram_parameter("out", [C_OUT, R_OUT, W], dt.bfloat16,
                                      isOutput=True)
    dbg = {}
    if debug_out:
        for nm, shp, dty in [
                ("d_hm", [64, 1 + R_HM * XC_W + 1], dt.bfloat16),
                ("d_offmask", [41, HW_CMB], dt.bfloat16),
                ("d_valT", [128, NS], dt.bfloat16),
                ("d_main0", [128, 3 + R_CMB * CMB_W + 3], dt.bfloat16),
                ("d_main1", [128, 3 + R_CMB * CMB_W + 3], dt.bfloat16),
                ("d_contour", [128, 3 + R_CMB * CMB_W + 3], dt.bfloat16),
                ("d_attn", [1, ACMB], dt.bfloat16)]:
            dbg[nm] = nc.declare_dram_parameter(nm, shp, dty,
                                                isOutput=True)

    with tile.TileContext(nc) as tc:
        with (
            tc.tile_pool(name="const", bufs=1) as const,
            tc.tile_pool(name="work", bufs=1) as work,
            tc.tile_pool(name="gath", bufs=6) as gath,
            tc.tile_pool(name="ps_m", bufs=7, space="PSUM") as ps_m,
            tc.tile_pool(name="ps_t", bufs=1, space="PSUM") as ps_t,
        ):
            _lp = tc.For_i(0, loop_n) if loop_n > 1 else \
                contextlib.nullcontext()
            with _lp:
                # ---------- load constants ----------
                def load(dram):
                    t = const.tile(list(dram.shape), dram.dtype,
                                   tag=dram.name + "_c", name=dram.name + "_c")
                    nc.sync.dma_start(t[:], dram[:])
                    return t

                xc = load(xc_d)
                w1 = load(w1_d)
                s1 = load(s1_d)
                b1 = load(b1_d)
                w2 = load(w2_d)
                b2 = load(b2_d)
                pyb = load(pyb_d)
                pxb = load(pxb_d)
                w2e = load(w2e_d)
                dcb = load(dcb_d)
                wdw = load(wdw_d)
                sdw = load(sdw_d)
                bdw = load(bdw_d)
                wpw = load(wpw_d)
                bpw = load(bpw_d)
                saw = load(saw_d)
                fuw = load(fuw_d)
                sf = load(sf_d)
                bf = load(bf_d)
                ones = load(ones_d)
                mhm = load(mhm_d)
                mcmb = load(mcmb_d)

                identb = const.tile([128, 128], dt.bfloat16, tag="identb")
                make_identity(nc, identb[:])
                identf = const.tile([128, 128], dt.float32, tag="identf")
                make_identity(nc, identf[:])

                # persistent PSUM tile helper for transposes (tag-shared
                # with conv chunk psums; rotates in ps_a's 2 bufs)
                def ps_cv():
                    return ps_m.tile([128, 512], dt.float32, tag="mm",
                                     name="ps_mm")

                def ps_tp():
                    return ps_t.tile([128, 512], dt.bfloat16, tag="tp",
                                     name="ps_tp")

                if stage >= 1:
                    # ---------- conv1: x -> h|hm (64ch, R_HM rows) ----------
                    # flat-contiguous rhs over the padded grid (strided matmul
                    # rhs hangs HW); pad columns compute garbage that
                    # epilogues skip.
                    xcf = xc[:]
                    hm = work.tile([64, 1 + R_HM * XC_W + 1], dt.bfloat16,
                                   tag="hm")
                    nc.gpsimd.memset(hm[:], 0.0)

                    def conv3x3(src_flat, src_w, lhsT_of, prow0, prow1, epi):
                        # out position p = prow*src_w + col (all cols); rhs
                        # flat slice = src_flat[, p + (ky-1)*src_w + (kx-1)]
                        # (+1 pad); chunk-pairs share one weight load per tap
                        chunks = []
                        r = prow0
                        while r < prow1:
                            nr = min(7, prow1 - r)
                            chunks.append((r, nr))
                            r += nr
                        pairs = [chunks[i:i + 2]
                                 for i in range(0, len(chunks), 2)]
                        for pair in pairs:
                            pss = [ps_cv() for _ in pair]
                            for t in range(KK):
                                ky, kx = t // 3, t % 3
                                for ci, (r0p, nr) in enumerate(pair):
                                    n = nr * src_w
                                    s0 = 1 + (r0p + ky - 1) * src_w \
                                        + (kx - 1)
                                    nc.tensor.matmul(
                                        pss[ci][:lhsT_of(t).shape[-1], :n],
                                        lhsT_of(t),
                                        src_flat[:, s0:s0 + n],
                                        start=(t == 0), stop=(t == KK - 1))
                            for ci, (r0p, nr) in enumerate(pair):
                                epi(r0p, nr, pss[ci])

                    def epi1(r0p, nr, ps):
                        base = 1 + (r0p - 1) * XC_W
                        nc.scalar.activation(
                            hm[:, base:base + nr * XC_W]
                            .rearrange("p (a b) -> p a b", b=XC_W)[:, :, 1:65],
                            ps[0:64, 0:nr * XC_W]
                            .rearrange("p (a b) -> p a b", b=XC_W)[:, :, 1:65],
                            Act.Relu, bias=b1[:], scale=s1[:])

                    conv3x3(xcf, XC_W, lambda t: w1[:, t, :], 1, R_X - 1, epi1)

                    # zero invalid halo rows (only ever in the top-4/bottom-4)
                    hmv = hm[:, 1:1 + R_HM * XC_W]\
                        .rearrange("p (a b) -> p a b", b=XC_W)[:, :, 1:65]
                    nc.vector.tensor_tensor(
                        hmv[:, 0:4], hmv[:, 0:4],
                        mhm[:, 0:4].unsqueeze(-1).broadcast_to([64, 4, 64]),
                        Alu.mult)
                    nc.vector.tensor_tensor(
                        hmv[:, R_HM - 4:R_HM], hmv[:, R_HM - 4:R_HM],
                        mhm[:, 4:8].unsqueeze(-1).broadcast_to([64, 4, 64]),
                        Alu.mult)

                    # ---------- conv2 -> offsets(18)|mask(9), R_CMB rows -----
                    # 48 partitions (xbar-transpose wants a multiple of 16);
                    # rows 41-47 are unused garbage
                    offmask = work.tile([48, HW_CMB], dt.bfloat16,
                                        tag="offmask")
                    nc.gpsimd.memset(offmask[32:48, :], 0.0)

                    def epi2(r0p, nr, ps):
                        j0 = r0p - 1          # offset-row index
                        pv = ps[:, 0:nr * XC_W]\
                            .rearrange("p (a b) -> p a b", b=XC_W)
                        nc.scalar.activation(
                            offmask[0:41, j0 * 64:(j0 + nr) * 64]
                            .rearrange("p (a b) -> p a b", b=64),
                            pv[0:41, :, 1:65],
                            Act.Identity, bias=b2[:], scale=1.0)

                    conv3x3(hm[:], XC_W, lambda t: w2[:, t, :], 1, 1 + R_CMB,
                            epi2)

                    # ---------- transpose offsets/mask to hw-major ----------
                    # xbar DMA transposes: toffs[p, g, c] =
                    # offmask[c, g*128 + p]; split so groups 0-6 (ready
                    # after ~2/6 of conv2's rows) unblock the first
                    # gathers early
                    toffs = work.tile([128, NGK, 48], dt.bfloat16,
                                      tag="toffs")
                    nc.sync.dma_start_transpose(toffs[:, 0:7, :],
                                                offmask[:, 0:7 * 128])
                    nc.sync.dma_start_transpose(toffs[:, 7:NGK, :],
                                                offmask[:, 7 * 128:HW_CMB])

                    def tof(c0, c1, step=1):
                        # [128, 9-ish, NGK] kk-major view of toffs cols
                        return toffs[:, :, c0:c1:step]\
                            .rearrange("p a b -> p b a")

                    offy = tof(0, 18, 2)            # [128, 9, NGK]
                    offx = tof(1, 18, 2)

                    # ---------- bilinear fields (hw-major) ----------
                    # index math first (unblocks the gathers), weights
                    # after (they overlap the first gathers)
                    fsh = [128, KK, NGK]

                    def ftile(name):
                        return work.tile(fsh, dt.float32, tag=name, name=name)

                    # tile buffers are reused across lifetimes via shared
                    # tags (work pool bufs=1 -> same buffer, WAR-tracked):
                    # fy reuses qf's, fx reuses parf's, ta py's, tb px's.
                    py, px = ftile("py"), ftile("px")
                    y0f, x0f = ftile("y0f"), ftile("x0f")
                    tu = ftile("tu")
                    qf, parf = ftile("qf"), ftile("parf")
                    # corner weights: [kk][g][corner4: 00,10,01,11][dup-pair]
                    wsb = work.tile([128, KK, NGK, 4, 2], dt.bfloat16,
                                    tag="wsb")

                    # floor via fp-add magic: rint(v) = (v + 2^23) - 2^23 in
                    # f32 (round-nearest-even, identical on HW and in numpy);
                    # floor(py) = rint(py - 0.5) for py >= 0.  The int16 cast
                    # then converts an exact integer (rounding-mode-proof).
                    # The whole index chain runs per group-half (0:7 /
                    # 7:19) so the first gathers can launch while conv2's
                    # later rows are still in flight.
                    MAGIC = float(1 << 23)
                    idxwA = work.tile([128, KK, 7, 8], dt.int16, tag="idxwA")
                    idxwB = work.tile([128, KK, NGK - 7, 8], dt.int16,
                                      tag="idxwB")
                    for (a, b, idxw) in [(0, 7, idxwA), (7, NGK, idxwB)]:
                        def S(t):
                            return t[:, :, a:b]
                        nc.vector.tensor_tensor(S(py), offy[:, :, a:b],
                                                S(pyb), Alu.add)
                        nc.vector.tensor_scalar(S(py), S(py), 0.0,
                                                float(SLAB_R - 2),
                                                Alu.max, Alu.min)
                        nc.vector.tensor_scalar(S(tu), S(py), 0.5, MAGIC,
                                                Alu.subtract, Alu.add)
                        nc.vector.tensor_scalar(S(y0f), S(tu), MAGIC, None,
                                                Alu.subtract)

                        nc.vector.tensor_tensor(S(px), offx[:, :, a:b],
                                                S(pxb), Alu.add)
                        nc.vector.tensor_scalar(S(px), S(px), 0.0,
                                                float(SLAB_W - 2),
                                                Alu.max, Alu.min)
                        nc.vector.tensor_scalar(S(tu), S(px), 0.5, MAGIC,
                                                Alu.subtract, Alu.add)
                        nc.vector.tensor_scalar(S(x0f), S(tu), MAGIC, None,
                                                Alu.subtract)

                        # pair-slab entry index:
                        #   q = floor(y0/2) = rint(y0*0.5 - 0.25)  (exact
                        #       for integer y0; 0.25 keeps half-integers
                        #       off the round-to-even boundary)
                        #   parity = y0 - 2q
                        #   eidx = (q + parity*1728/72)*72 + x0
                        nc.vector.tensor_scalar(S(tu), S(y0f), 0.5, 0.25,
                                                Alu.mult, Alu.subtract)
                        nc.vector.tensor_scalar(S(qf), S(tu), MAGIC, MAGIC,
                                                Alu.add, Alu.subtract)
                        nc.vector.scalar_tensor_tensor(
                            S(parf), S(qf), -2.0, S(y0f), Alu.mult, Alu.add)
                        nc.vector.scalar_tensor_tensor(
                            S(tu), S(parf), float(N_PAIR_E // SLAB_W),
                            S(qf), Alu.mult, Alu.add)
                        idxf_h = work.tile([128, KK, b - a], dt.float32,
                                           tag=f"idxf{a}")
                        nc.vector.scalar_tensor_tensor(
                            idxf_h[:], S(tu), float(SLAB_W), S(x0f),
                            Alu.mult, Alu.add)

                        # wrap + replicate indices for dma_gather:
                        # idxw[16r+p16, kk, g, q] = idx of sample g*128 +
                        # 16q + p16; the gather's j-th index lives at
                        # partition j%16, free j//16.  The partition shift
                        # runs on the PE (f32 identity-column matmul is
                        # exact on these integers) -- the DMA version
                        # costs ~1k 2-byte descriptors per copy on HW.
                        n_h = KK * (b - a)
                        for q in range(8):
                            ps = ps_cv()
                            nc.tensor.matmul(
                                ps[0:16, :n_h],
                                identf[:, 16 * q:16 * (q + 1)],
                                idxf_h[:].rearrange("p a b -> p (a b)"),
                                start=True, stop=True)
                            nc.vector.tensor_copy(
                                idxw[0:16, :, :, q],
                                ps[0:16, :n_h]
                                .rearrange("p (a b) -> p a b", b=b - a))
                        nc.sync.dma_start(idxw[16:32], idxw[0:16])
                        nc.sync.dma_start(idxw[32:64], idxw[0:32])
                        nc.sync.dma_start(idxw[64:128], idxw[0:64])

                    # fractional parts + mask (overlap the first gathers)
                    fy, fx = ftile("qf"), ftile("parf")
                    nc.vector.tensor_tensor(fy[:], py[:], y0f[:],
                                            Alu.subtract)
                    nc.vector.tensor_tensor(fx[:], px[:], x0f[:],
                                            Alu.subtract)
                    msk2 = work.tile([128, KK, NGK], dt.float32, tag="msk2")
                    nc.scalar.activation(msk2[:], tof(32, 41), Act.Sigmoid)
                    maskT = msk2[:]

                    # corner weights (mask folded), order (00, 10, 01, 11)
                    # matching the gathered pair-slab patch layout
                    # [v00 v10 v01 v11]: w00=m(1-fy)(1-fx), w10=m*fy(1-fx),
                    # w01=m(1-fy)fx, w11=m*fy*fx.  Each weight is stored as
                    # an adjacent duplicated pair so the mul's weight operand
                    # has a packed last dim (stride 1, count 2) -> DVE 2x
                    # mode applies despite the broadcast.
                    ta, tb = ftile("py"), ftile("px")
                    nc.vector.tensor_tensor(tb[:], maskT, fy[:], Alu.mult)
                    nc.vector.tensor_tensor(ta[:], maskT, tb[:],
                                            Alu.subtract)
                    nc.vector.tensor_tensor(tu[:], ta[:], fx[:], Alu.mult)
                    for j in range(2):
                        nc.vector.tensor_copy(wsb[:, :, :, 2, j], tu[:])
                    nc.vector.tensor_tensor(tu[:], ta[:], tu[:],
                                            Alu.subtract)
                    for j in range(2):
                        nc.vector.tensor_copy(wsb[:, :, :, 0, j], tu[:])
                    nc.vector.tensor_tensor(tu[:], tb[:], fx[:], Alu.mult)
                    for j in range(2):
                        nc.vector.tensor_copy(wsb[:, :, :, 3, j], tu[:])
                    nc.vector.tensor_tensor(tu[:], tb[:], tu[:],
                                            Alu.subtract)
                    for j in range(2):
                        nc.vector.tensor_copy(wsb[:, :, :, 1, j], tu[:])

                    def cmb_grid(t):      # [128, R_CMB, CMB_W] view of flat
                        return t[:, 3:3 + R_CMB * CMB_W]\
                            .rearrange("p (a b) -> p a b", b=CMB_W)

                    mm_chunks = [(0, 512), (512, 512), (1024, 512),
                                 (1536, 512), (2048, 384)]

                if stage >= 2:
                    # ---------- contour branch ----------
                    hc = work.tile([C_IN, HW_CMB], dt.bfloat16, tag="hc")

                    def epi_dw(r0p, nr, ps):
                        j0 = r0p - 2
                        nc.scalar.activation(
                            hc[:, j0 * 64:(j0 + nr) * 64]
                            .rearrange("p (a b) -> p a b", b=64),
                            ps[:, 0:nr * XC_W]
                            .rearrange("p (a b) -> p a b", b=XC_W)[:, :, 1:65],
                            Act.Relu, bias=bdw[:], scale=sdw[:])

                    conv3x3(xcf, XC_W, lambda t: wdw[:, t, :], 2, 2 + R_CMB,
                            epi_dw)
                    contour = work.tile([C_IN, 3 + R_CMB * CMB_W + 3],
                                        dt.bfloat16, tag="contour")
                    nc.gpsimd.memset(contour[:], 0.0)
                    for (c0, cn) in mm_chunks:
                        ps = ps_cv()
                        nc.tensor.matmul(ps[:, :cn], wpw[:], hc[:, c0:c0 + cn],
                                         start=True, stop=True)
                        r0, nr = c0 // 64, cn // 64
                        nc.scalar.activation(
                            cmb_grid(contour)[:, r0:r0 + nr, 3:67],
                            ps[:, :cn].rearrange("p (a b) -> p a b", b=64),
                            Act.Identity, bias=bpw[:], scale=1.0)

                if stage >= 3:
                    # ---------- per-kk: gather, weight, fold, transpose,
                    # ---------- and half-0 einsum (kk-major) ----------
                    xsp_flat = AP(tensor=xsp_d, offset=0,
                                  ap=[[2 * C_IN, N_ENT], [1, 4 * C_IN]])
                    valT = work.tile([128, 1, NS], dt.bfloat16, tag="valT")

                    main_sb = []
                    for hf in range(2):
                        m_t = work.tile([128, 3 + R_CMB * CMB_W + 3],
                                        dt.bfloat16, tag=f"main{hf}")
                        nc.gpsimd.memset(m_t[:], 0.0)
                        main_sb.append(m_t)

                    # persistent kk-major einsum accumulators: half 0 all 5
                    # chunks + half 1 chunks 0-1 (7 of the 7 ps_m bufs);
                    # half-1 chunks 2-4 run post-loop.
                    ek = [(0, c0, cn) for (c0, cn) in mm_chunks] + \
                        [(1, c0, cn) for (c0, cn) in mm_chunks[:2]]
                    pse0 = {}
                    for (hf, c0, cn) in ek:
                        pse0[(hf, c0)] = ps_m.tile([128, 512], dt.float32,
                                                   tag="mm", name="mm")

                    # <=1024 descriptors per gather: the 16KB SWDGE
                    # descriptor carveout (16B/desc) caps one instruction
                    # at ~1024 on HW.  Splits align with the idxwA/idxwB
                    # group halves.
                    splits = [(0, 7, 0), (7, 6, 0), (13, 6, 6)]
                    for kk_i in range(KK):
                        halves = []
                        for gi, (g0, ng, grel) in enumerate(splits):
                            idxw = idxwA if g0 < 7 else idxwB
                            gtb = gath.tile([128, 7, 512], dt.bfloat16,
                                            tag="gtb")
                            halves.append((g0, ng, gtb))
                            ni = ng * 128
                            nc.gpsimd.dma_gather(
                                gtb[:, 0:ng, :], xsp_flat,
                                idxw[:, kk_i, grel:grel + ng, :], ni, ni,
                                elem_size=512, elem_step=2 * C_IN,
                                queue_num=(3 * kk_i + gi) % 4)

                        # corner weights in place (paired dup weights ->
                        # packed last dim -> DVE 2x), then fold the 4
                        # corners with 2 tree adds (packed, 2x)
                        for (g0, ng, gtb) in halves:
                            g_v = gtb[:, 0:ng, :]\
                                .rearrange("p a (b c d) -> p a b c d",
                                           b=4, d=2)
                            w_v = wsb[:, kk_i, g0:g0 + ng].unsqueeze(3)\
                                .broadcast_to([128, ng, 4, 64, 2])
                            nc.vector.tensor_tensor(g_v, g_v, w_v, Alu.mult)
                            nc.vector.tensor_tensor(
                                gtb[:, 0:ng, 0:256], gtb[:, 0:ng, 0:256],
                                gtb[:, 0:ng, 256:512], Alu.add)
                            nc.vector.tensor_tensor(
                                gtb[:, 0:ng, 0:128], gtb[:, 0:ng, 0:128],
                                gtb[:, 0:ng, 128:256], Alu.add)

                        # transpose each 128-sample group to channel-major
                        for gch in range(5):      # 4+4+4+4+3 groups of 128
                            nu = 4 if gch < 4 else 3
                            pst = ps_tp()
                            for u in range(nu):
                                g = gch * 4 + u
                                for (g0, ng, t_) in halves:
                                    if g0 <= g < g0 + ng:
                                        gtb = t_
                                        gl = g - g0
                                        break
                                nc.tensor.transpose(
                                    pst[:, u * 128:(u + 1) * 128],
                                    gtb[:, gl, 0:128], identb[:])
                            nc.scalar.activation(
                                valT[:, 0, kk_i * HW_CMB + gch * 512:
                                     kk_i * HW_CMB + gch * 512 + nu * 128],
                                pst[:, :nu * 128], Act.Copy)

                        # in-loop einsum contributions of this kk
                        for (hf, c0, cn) in ek:
                            rhs = valT[:, 0, kk_i * HW_CMB + c0:
                                       kk_i * HW_CMB + c0 + cn]
                            nc.tensor.matmul(
                                pse0[(hf, c0)][:, :cn],
                                w2e[:, kk_i, hf * 128:(hf + 1) * 128], rhs,
                                start=(kk_i == 0), stop=(kk_i == KK - 1))

                    def epi_main(hf, c0, cn, ps):
                        r0, nr = c0 // 64, cn // 64
                        nc.scalar.activation(
                            cmb_grid(main_sb[hf])[:, r0:r0 + nr, 3:67],
                            ps[:, :cn].rearrange("p (a b) -> p a b", b=64),
                            Act.Identity, bias=dcb[:, hf:hf + 1], scale=1.0)

                    for (hf, c0, cn) in ek:
                        epi_main(hf, c0, cn, pse0[(hf, c0)])

                    # ---------- remaining einsum: half-1 chunks 2-4 ----------
                    # emission deferred (stage>=5 weaves it between the
                    # first attention chunks, which only need in-loop
                    # einsum results)
                    def rest_einsum():
                        rest = [(1, c0, cn) for (c0, cn) in mm_chunks[2:]]
                        psr = {}
                        for (hf, c0, cn) in rest:
                            psr[(hf, c0)] = ps_m.tile([128, 512],
                                                      dt.float32,
                                                      tag="mm", name="mm")
                        for kk_i in range(KK):
                            for (hf, c0, cn) in rest:
                                rhs = valT[:, 0, kk_i * HW_CMB + c0:
                                           kk_i * HW_CMB + c0 + cn]
                                nc.tensor.matmul(
                                    psr[(hf, c0)][:, :cn],
                                    w2e[:, kk_i, hf * 128:(hf + 1) * 128],
                                    rhs, start=(kk_i == 0),
                                    stop=(kk_i == KK - 1))
                        for (hf, c0, cn) in rest:
                            epi_main(hf, c0, cn, psr[(hf, c0)])

                    # ---------- zero invalid rows (only top-3/bottom-3) -----
                    cmb = [main_sb[0], main_sb[1], contour]

                    def zero_rows(r0z, msl):
                        for cti in range(3):
                            ctv = cmb_grid(cmb[cti])[:, :, 3:67]
                            nc.vector.tensor_tensor(
                                ctv[:, r0z:r0z + 3], ctv[:, r0z:r0z + 3],
                                mcmb[:, msl:msl + 3].unsqueeze(-1)
                                .broadcast_to([128, 3, 64]), Alu.mult)

                    zero_rows(0, 0)           # top rows: in-loop chunks
                    if stage < 5:
                        rest_einsum()
                        zero_rows(R_CMB - 3, 3)

                if stage >= 5:
                    # ---------- attention: 7x7 conv -> 1 channel ----------
                    # pm49[dx*7+dy, j*70+c'] = sum_c saw[c,kt,dx*7+dy]*cmb
                    # over kt blocks; then fold dx (PE, shifted), fold dy
                    # (PE, K=1 accumulated matmuls on contiguous windows),
                    # sigmoid.  Emission is software-pipelined per chunk so
                    # the 4 PE stages and their ACT copies overlap.
                    NPM = R_CMB * CMB_W                     # 2660
                    NP7 = NPM - 6
                    pm49 = work.tile([49, NPM], dt.bfloat16, tag="pm49")
                    pm7 = work.tile([7, NPM], dt.bfloat16, tag="pm7")
                    nc.gpsimd.memset(pm7[:, NP7:NPM], 0.0)
                    attn = work.tile([1, ACMB], dt.bfloat16, tag="attn")
                    attn_r = work.tile([128, ACMB], dt.bfloat16,
                                       tag="attn_r")
                    a_chunks = [(0, 448), (448, 448), (896, 448),
                                (1344, 448), (1792, 448), (2240, 420)]
                    x_chunks = [(0, 448), (448, 448), (896, 448), (1344, 448),
                                (1792, 448), (2240, NP7 - 2240)]
                    f_chunks = [(0, 448), (448, 448), (896, 448),
                                (1344, 448), (1792, 448)]

                    def do_pm49(i0, n):
                        ps = ps_cv()
                        for kt in range(3):
                            nc.tensor.matmul(ps[0:49, :n], saw[:, kt, :],
                                             cmb[kt][:, 3 + i0:3 + i0 + n],
                                             start=(kt == 0), stop=(kt == 2))
                        nc.scalar.activation(pm49[:, i0:i0 + n],
                                             ps[0:49, :n], Act.Copy)

                    def do_dx(i0, n):
                        # pm7[dy, q] = sum_dx pm49[dx*7+dy, q+dx]
                        ps = ps_cv()
                        for dx in range(7):
                            nc.tensor.matmul(
                                ps[0:7, :n], identb[0:49, 7 * dx:7 * dx + 7],
                                pm49[:, i0 + dx:i0 + dx + n],
                                start=(dx == 0), stop=(dx == 6))
                        nc.scalar.activation(pm7[:, i0:i0 + n], ps[0:7, :n],
                                             Act.Copy)

                    def do_dy(i0, n):
                        # attn[q=r*70+c'] = sig(sum_dy pm7[dy, q + dy*70])
                        ps = ps_cv()
                        for dy in range(7):
                            nc.tensor.matmul(
                                ps[0:1, :n], identb[0:7, dy:dy + 1],
                                pm7[0:7, dy * CMB_W + i0:dy * CMB_W + i0 + n],
                                start=(dy == 0), stop=(dy == 6))
                        nc.scalar.activation(attn[:, i0:i0 + n], ps[0:1, :n],
                                             Act.Sigmoid)

                    def do_rep(i0, n):
                        # replicate attn to 128 partitions via K=1 matmul
                        ps = ps_cv()
                        nc.tensor.matmul(ps[:, :n], ones[0:1, :],
                                         attn[:, i0:i0 + n],
                                         start=True, stop=True)
                        nc.scalar.activation(attn_r[:, i0:i0 + n], ps[:, :n],
                                             Act.Copy)

                    # fusion 1x1 matmuls are independent of attn: woven in
                    # to keep the PE busy during the fold ACT copies
                    fvts = []
                    fu_work = []
                    if stage >= 6:
                        for hf in range(2):
                            fvt = work.tile([128, ACMB], dt.bfloat16,
                                            tag=f"fvt{hf}")
                            fvts.append(fvt)

                        def do_fu(hf, i0, n):
                            ps = ps_cv()
                            for kt in range(3):
                                rhs = cmb[kt][:, 3 + 3 * CMB_W + i0:
                                              3 + 3 * CMB_W + i0 + n]
                                nc.tensor.matmul(
                                    ps[:, :n],
                                    fuw[:, kt, hf * 128:(hf + 1) * 128],
                                    rhs, start=(kt == 0), stop=(kt == 2))
                            nc.scalar.activation(fvts[hf][:, i0:i0 + n],
                                                 ps[:, :n], Act.Copy)

                        fu_work = [(hf, i0, n) for hf in range(2)
                                   for (i0, n) in f_chunks]

                    # software-pipelined emission: pm49 chunks 0-1 only
                    # need in-loop einsum output, so they run BEFORE the
                    # remaining einsum chunks; then dx chunk i needs pm49
                    # through chunk i+1, dy chunk i needs pm7 through
                    # chunk i+1, rep chunk i needs dy chunk i.
                    do_pm49(*a_chunks[0])
                    do_pm49(*a_chunks[1])
                    rest_einsum()
                    zero_rows(R_CMB - 3, 3)
                    prog = [("dx", 0), ("pm49", 2), ("fu", 0), ("dx", 1),
                            ("pm49", 3), ("dy", 0), ("fu", 1), ("pm49", 4),
                            ("dx", 2), ("dy", 1), ("rep", 0), ("fu", 2),
                            ("pm49", 5), ("dx", 3), ("dy", 2), ("rep", 1),
                            ("fu", 3), ("dx", 4), ("dy", 3), ("rep", 2),
                            ("fu", 4), ("dx", 5), ("dy", 4), ("rep", 3),
                            ("fu", 5), ("rep", 4)]
                    nfu = len(fu_work)
                    for it in prog:
                        kind, arg = it
                        if kind == "pm49":
                            do_pm49(*a_chunks[arg])
                        elif kind == "dx":
                            do_dx(*x_chunks[arg])
                        elif kind == "dy":
                            do_dy(*f_chunks[arg])
                        elif kind == "rep":
                            do_rep(*f_chunks[arg])
                        elif kind == "fu":
                            if arg < nfu:
                                do_fu(*fu_work[arg])
                    for w_ in fu_work[6:]:
                        do_fu(*w_)

                if stage >= 6:
                    # ---------- fusion epilogue: attn multiply + out --------
                    for hf in range(2):
                        fvt = fvts[hf]
                        # attn multiply: both in 70-col layout, 3-col offset
                        fm = work.tile([128, R_OUT, 64], dt.bfloat16,
                                       tag="fm")
                        nc.vector.tensor_tensor(
                            fm[:],
                            fvt[:].rearrange("p (a b) -> p a b",
                                             b=CMB_W)[:, :, 3:67],
                            attn_r[:].rearrange("p (a b) -> p a b",
                                                b=CMB_W)[:, :, 0:64],
                            Alu.mult)
                        outt = work.tile([128, R_OUT * W], dt.bfloat16,
                                         tag="outt")
                        nc.scalar.activation(
                            outt[:].rearrange("p (a b) -> p a b", b=64),
                            fm[:], Act.Relu, bias=bf[:, hf:hf + 1],
                            scale=sf[:, hf:hf + 1])
                        nc.sync.dma_start(
                            out_d[hf * 128:(hf + 1) * 128, :, :],
                            outt[:].rearrange("p (a b) -> p a b", b=64))

                if stage < 6:
                    o = work.tile([128, 16], dt.bfloat16, tag="stub")
                    nc.gpsimd.memset(o[:], 0.0)
                    for hf in range(2):
                        nc.sync.dma_start(
                            out_d[hf * 128:(hf + 1) * 128, 0:1, 0:16],
                            o[:].rearrange("p (a b) -> p a b", b=16))

                if debug_out:
                    def dump(nm, ap):
                        if len(ap.shape) > 2:
                            ap = ap.rearrange("p ... -> p (...)")
                        nc.sync.dma_start(dbg[nm][:], ap)
                    dump("d_hm", hm[:])
                    dump("d_offmask", offmask[:])
                    dump("d_valT", valT[:])
                    dump("d_main0", main_sb[0][:])
                    dump("d_main1", main_sb[1][:])
                    dump("d_contour", contour[:])
                    dump("d_attn", attn[:])

    nc.compile()
    return nc


# ---------------- host-side input prep ----------------

def prep_core_inputs(d, core_id):
    b, half = core_id // 2, core_id % 2
    r0 = half * R_OUT

    x = _f32(d["x"][b])                       # [C_IN, H, W]

    xcg = np.zeros((C_IN, R_X, XC_W), np.float32)
    lo, hi = r0 - 5, r0 + R_OUT + 5
    slo, shi = max(lo, 0), min(hi, H)
    xcg[:, slo - lo:shi - lo, 1:65] = x[:, slo:shi, :]
    xc = np.zeros((C_IN, 1 + R_X * XC_W + 1), np.float32)
    xc[:, 1:1 + R_X * XC_W] = xcg.reshape(C_IN, -1)

    # pair-interleaved slab: entry (pair p, col c) holds the two pixels
    # (2p + parity, c), (2p + 1 + parity, c) with 128ch each; even copy
    # (parity 0) at entries [0, 1728), odd copy at [1728, 3456).
    slab = np.zeros((SLAB_R + 1, SLAB_W, C_IN), np.float32)
    lo2, hi2 = r0 - 8, r0 + R_OUT + 8
    slo2, shi2 = max(lo2, 0), min(hi2, H)
    slab[slo2 - lo2:shi2 - lo2, 4:68, :] = \
        x[:, slo2:shi2, :].transpose(1, 2, 0)
    xsp = np.zeros((N_ENT + 1, 2 * C_IN), np.float32)
    ev = xsp[:N_PAIR_E].reshape(SLAB_R // 2, SLAB_W, 2, C_IN)
    ev[:, :, 0] = slab[0:SLAB_R:2, :, :]
    ev[:, :, 1] = slab[1:SLAB_R + 1:2, :, :]
    od = xsp[N_PAIR_E:N_ENT].reshape(SLAB_R // 2, SLAB_W, 2, C_IN)
    od[:, :, 0] = slab[1:SLAB_R:2, :, :]
    od[:, :, 1] = slab[2:SLAB_R + 1:2, :, :]

    w1 = np.zeros((C_IN, KK, 64), np.float32)
    for t in range(KK):
        ky, kx = t // 3, t % 3
        w1[:, t, 0:32] = d["oc1_w"][:, :, ky, kx].T
        w1[:, t, 32:64] = d["mc1_w"][:, :, ky, kx].T
    sc_o = d["obn_g"] / np.sqrt(d["obn_v"] + EPS)
    bi_o = (d["oc1_b"] - d["obn_m"]) * sc_o + d["obn_b"]
    sc_m = d["mbn_g"] / np.sqrt(d["mbn_v"] + EPS)
    bi_m = (d["mc1_b"] - d["mbn_m"]) * sc_m + d["mbn_b"]
    s1 = np.concatenate([sc_o, sc_m])[:, None]
    b1 = np.concatenate([bi_o, bi_m])[:, None]

    w2 = np.zeros((64, KK, 41), np.float32)
    for t in range(KK):
        ky, kx = t // 3, t % 3
        w2[0:32, t, 0:18] = d["oc2_w"][:, :, ky, kx].T
        w2[32:64, t, 32:41] = d["mc2_w"][:, :, ky, kx].T
    b2 = np.zeros((41, 1), np.float32)
    b2[0:18, 0] = d["oc2_b"]
    b2[32:41, 0] = d["mc2_b"]

    kk = np.arange(KK)
    hw = np.arange(HW_CMB)
    r_i, w_i = hw // 64, hw % 64
    pyb = (r_i[None, :] + 4 + (kk // 3)[:, None]).astype(np.float32)
    pxb = (w_i[None, :] + 3 + (kk % 3)[:, None]).astype(np.float32)
    pyb = pyb.reshape(KK, NGK, 128).transpose(2, 0, 1)
    pxb = pxb.reshape(KK, NGK, 128).transpose(2, 0, 1)

    w2e = d["dc_w"].reshape(C_OUT, C_IN, KK).transpose(1, 2, 0)

    wdw = np.zeros((C_IN, KK, C_IN), np.float32)
    for t in range(KK):
        ky, kx = t // 3, t % 3
        np.fill_diagonal(wdw[:, t, :], d["cb_dw_w"][:, 0, ky, kx])
    sc_c = d["cbn_g"] / np.sqrt(d["cbn_v"] + EPS)
    bi_c = (d["cb_dw_b"] - d["cbn_m"]) * sc_c + d["cbn_b"]

    wpw = d["cb_pw_w"][:, :, 0, 0].T
    # saw[c, kt, dx*7+dy] = sa_w[0, kt*128+c, dy, dx]
    saw = d["sa_w"][0].reshape(3, 128, 7, 7).transpose(1, 0, 3, 2)\
        .reshape(128, 3, 49)
    fuw = d["fu_w"][:, :, 0, 0].T.reshape(3, 128, C_OUT).transpose(1, 0, 2)
    sc_f = d["fbn_g"] / np.sqrt(d["fbn_v"] + EPS)
    bi_f = (d["fu_b"] - d["fbn_m"]) * sc_f + d["fbn_b"]

    rows_hm = np.arange(r0 - 4, r0 + R_OUT + 4)
    vhm = ((rows_hm >= 0) & (rows_hm < H)).astype(np.float32)
    mhm = np.broadcast_to(np.concatenate([vhm[0:4], vhm[-4:]]),
                          (64, 8)).copy()
    rows_cmb = np.arange(r0 - 3, r0 + R_OUT + 3)
    vcmb = ((rows_cmb >= 0) & (rows_cmb < H)).astype(np.float32)
    mcmb = np.broadcast_to(np.concatenate([vcmb[0:3], vcmb[-3:]]),
                           (128, 6)).copy()

    return {
        "xc": _bf16(xc), "xsp": _bf16(xsp),
        "w1": _bf16(w1), "s1": _f32(s1), "b1": _f32(b1),
        "w2": _bf16(w2), "b2": _f32(b2),
        "pyb": _f32(pyb), "pxb": _f32(pxb),
        "w2e": _bf16(w2e), "dcb": _f32(d["dc_b"].reshape(2, 128).T),
        "wdw": _bf16(wdw), "sdw": _f32(sc_c[:, None]),
        "bdw": _f32(bi_c[:, None]),
        "wpw": _bf16(wpw), "bpw": _f32(d["cb_pw_b"][:, None]),
        "saw": _bf16(saw), "fuw": _bf16(fuw),
        "sf": _f32(sc_f.reshape(2, 128).T), "bf": _f32(bi_f.reshape(2, 128).T),
        "ones1": _bf16(np.ones((7, 128), np.float32)),
        "mhm": _bf16(mhm), "mcmb": _bf16(mcmb),
    }


_NC_CACHE = {}


def get_nc():
    if "nc" not in _NC_CACHE:
        _NC_CACHE["nc"] = build_bass()
    return _NC_CACHE["nc"]


def kernel(**inputs):
    from concourse.bass_utils import run_bass_kernel_spmd

    nc = get_nc()
    d = {k: np.asarray(v) for k, v in inputs.items()}
    in_maps = [prep_core_inputs(d, c) for c in range(8)]
    res = run_bass_kernel_spmd(nc, in_maps, core_ids=list(range(8)))

    out = np.zeros((B, C_OUT, H, W), np.float32)
    for c in range(8):
        b, half = c // 2, c % 2
        out[b, :, half * R_OUT:(half + 1) * R_OUT, :] = \
            np.asarray(res.results[c]["out"], dtype=np.float32)
    return out
